# revision 29
# baseline (speedup 1.0000x reference)
"""CapsuleConv2d Trainium2 kernel.

Math: out[b,o,h,w,i,j] = sum_{ci,kh,kw} W[j,o,ci,kh,kw] * x[b,ci,h+kh-1,w+kw-1,i,0]
i.e. a 3x3 pad-1 conv with effective batch (b,i): 64 images [64,56,56],
Cout = 256 (co = j*64+o).

Strategy (8 cores, data-parallel over b):
  - each core takes 2 of 16 b-groups; the 4 ic0 images of a b-group ride in
    the free dim (w,i) so HBM loads are fully contiguous.
  - x lives in SBUF as [ci, h_pad, (w_pad, i)] with a zero halo; each conv
    offset (kh,kw) is a sliced matmul rhs, accumulated in PSUM over 9 offsets.
  - Cin=64 only fills half the 128-row PE array: the host stages x twice
    (channels 64..127 = copy), and the co-high half's matmuls run on
    partitions 64..127 concurrently (row tiling) -> 2x PE throughput.
  - fp16 x/W/y: halves DMA traffic and enables fast weight load (FWL).
  - G=2 h-tile groups per tap loop, 8 PSUM banks in flight.
  - group outputs staged into one SBUF tile and stored with a single large
    contiguous DMA; host does the final (b,o,h,w,i,j) layout transpose.
"""

import sys

if "/opt/trn_rl_repo" not in sys.path:
    sys.path.insert(0, "/opt/trn_rl_repo")

import numpy as np

NCORES = 8
B, C, H, W_, IC0, WC1, O = 16, 64, 56, 56, 4, 4, 64
CO = WC1 * O  # 256
BPC = B // NCORES  # b-groups per core
WI = W_ * IC0  # 224 = free-dim run per image row
HP, WP = H + 2, (W_ + 2) * IC0  # padded sbuf tile dims: 58, 232
NKER = 9
NHT = H // 2  # 28 psum tiles of 2 output rows each

_COMPILED = None
_CFG = {}


def _build(
    dt16=True,
    grp=2,
    split_copy=True,
    ldwopt=False,
    out16=True,
):
    import concourse.tile as tile
    from concourse import bacc, mybir

    dt = mybir.dt
    DT = dt.float16 if dt16 else dt.float32r
    ydt = dt.float16 if out16 else dt.float32

    if ldwopt:
        _patch_ldwopt()
    nc = bacc.Bacc(
        "TRN2", target_bir_lowering=False, debug=False, num_devices=NCORES
    )
    x_d = nc.dram_tensor("x", [BPC, 128, HP, WP], DT, kind="ExternalInput").ap()
    w_d = nc.dram_tensor("w", [128, NKER, 128], DT, kind="ExternalInput").ap()
    ngrp = NHT // grp
    nt = 2 * grp  # output tiles per group
    y_d = nc.dram_tensor(
        "y", [BPC, ngrp, 128, nt, 2, WI], ydt, kind="ExternalOutput"
    ).ap()

    with tile.TileContext(nc) as tc:
        with (
            tc.tile_pool(name="xp", bufs=1) as xp,
            tc.tile_pool(name="wp", bufs=1) as wp,
            tc.tile_pool(name="op", bufs=3) as op,
            tc.tile_pool(name="pp", bufs=8 // (2 * grp), space="PSUM") as pp,
        ):
            wt = wp.tile([128, NKER, 128], DT)
            # split the weight load so tap-0/1 weights land ASAP (first MMs
            # wait on them); w + b=1 chunks ride the Scalar HWDGE ring while
            # b=0 chunks ride Sync - the ~0.7us per-dma issue costs overlap
            nc.scalar.dma_start(wt[:, 0:2, :], w_d[:, 0:2, :])
            nc.scalar.dma_start(wt[:, 2:NKER, :], w_d[:, 2:NKER, :])

            # PE warmup: dummy matmuls on a zeroed junk tile. The HAM
            # clock-gate reaches 2.4 GHz ~8-9us after the first matmul
            # (A/B-verified: removing these costs ~1-3us), so an earlier
            # warmup start shifts the warm transition earlier; memset on
            # the otherwise-idle GpSimd engine unblocks the first dummy
            # sooner than the busy Vector engine would
            junk = wp.tile([128, 448], DT, tag="junk", name="junk")
            nc.gpsimd.memset(junk[:, :], 0.0)
            pwarm = pp.tile([128, 2, WI], mybir.dt.float32, tag="p00", name="pwarm")
            for _ in range(14):
                nc.tensor.matmul(
                    pwarm[:, :, :],
                    lhsT=junk[0:64, 0:128],
                    rhs=junk[0:64, :],
                    start=True,
                    stop=True,
                )

            xts = [
                xp.tile([128, HP, WP], DT, tag=f"x{b}", name=f"x{b}")
                for b in range(BPC)
            ]
            # only the first 6 rows of b=0 ride the sync ring in the head
            # window, so the chunk-0/weight DMA receipts land before the
            # warmup matmuls finish (receipts stretch from ~2us to ~5us when
            # bulk prefetch shares the window); rows 6-13 follow on the
            # scalar ring. 12 warmup matmuls hedge receipt jitter: a PE
            # idle hole between warmup and real matmuls costs its full
            # duration (A/B-tested: trimming to 8 dummies opened a 2.8us
            # hole on a slow-receipt run)
            for r0, r1 in zip(b0b := [0, 2, 6], b0b[1:]):
                nc.sync.dma_start(xts[0][0:128, r0:r1, :], x_d[0, :, r0:r1, :])
            nc.scalar.dma_start(xts[0][0:128, 6:14, :], x_d[0, :, 6:14, :])

            for b in range(BPC):
                xt = xts[b]
                for hg in range(ngrp):
                    ps = [
                        [
                            pp.tile(
                                [128, 2, WI],
                                dt.float32,
                                tag=f"p{half}{g}",
                                name=f"p{half}{g}",
                            )
                            for g in range(grp)
                        ]
                        for half in range(2)
                    ]
                    for k in range(NKER):
                        kh, kw = divmod(k, 3)
                        c0 = IC0 * kw
                        for g in range(grp):
                            h0 = 2 * (grp * hg + g)
                            for half in range(2):
                                p0 = 64 * half
                                nc.tensor.matmul(
                                    ps[half][g][:, :, :],
                                    lhsT=wt[p0 : p0 + 64, k, :],
                                    rhs=xt[
                                        p0 : p0 + 64,
                                        h0 + kh : h0 + kh + 2,
                                        c0 : c0 + WI,
                                    ],
                                    start=(k == 0),
                                    stop=(k == NKER - 1),
                                )
                    o = op.tile([128, nt, 2, WI], ydt, tag="o", name="o")
                    last = b == BPC - 1 and hg == ngrp - 1
                    for g in range(grp):
                        for half in range(2):
                            t = 2 * g + half
                            if split_copy and half == 1:
                                nc.scalar.copy(
                                    o[:, t, :, :], ps[half][g][:, :, :]
                                )
                            else:
                                nc.vector.tensor_copy(
                                    o[:, t, :, :], ps[half][g][:, :, :]
                                )
                            if last:
                                # split the final store so the kernel-end
                                # DMA receipt is small
                                eng = nc.sync if half == 0 else nc.scalar
                                eng.dma_start(
                                    y_d[b, hg, :, t, :, :], o[:, t, :, :]
                                )
                    if not last:
                        # alternate store issue between the two HWDGE rings
                        eng = nc.sync if hg % 2 == 0 else nc.scalar
                        eng.dma_start(y_d[b, hg, :, :, :, :], o[:, :, :, :])
                    if b == 0 and hg == 0:
                        # deferred bulk loads: b=0 tail rows first (needed
                        # from ~hg=3 on), then all of b=1 (needed at ~60us)
                        for r0, r1 in zip(db := [14, 18, 24, 41, HP], db[1:]):
                            nc.sync.dma_start(
                                xts[0][0:128, r0:r1, :], x_d[0, :, r0:r1, :]
                            )
                        for r0, r1 in zip(b1b := [0, 24, 41, HP], b1b[1:]):
                            nc.sync.dma_start(
                                xts[1][0:128, r0:r1, :], x_d[1, :, r0:r1, :]
                            )

    nc.compile()
    return nc


def _patch_ldwopt():
    import concourse.bass_utils as BU

    if getattr(BU, "_ldwopt_patched", False):
        return
    orig = BU.run_command

    def patched(argv, **kwargs):
        argv = [
            ("--enable-ldw-opt=true" if a == "--enable-ldw-opt=false" else a)
            for a in argv
        ]
        return orig(argv, **kwargs)

    BU.run_command = patched
    BU._ldwopt_patched = True


def _prep(x, W, dt16=True):
    x = np.asarray(x, dtype=np.float32)
    W = np.asarray(W, dtype=np.float32)
    npdt = np.float16 if dt16 else np.float32
    xs = x.reshape(B, C, H, WI)  # drop ic1, fuse (w,i)
    xpad = np.zeros((B, 128, HP, WP), npdt)  # host zero-pad = sbuf halo
    xpad[:, 0:64, 1 : H + 1, IC0 : IC0 + WI] = xs.astype(npdt)
    xpad[:, 64:128] = xpad[:, 0:64]  # duplicate for the co-high row half
    Wf = W.reshape(CO, C, 3, 3)
    Wt = np.ascontiguousarray(Wf.transpose(1, 2, 3, 0)).reshape(C, NKER, CO)
    wsb = np.ascontiguousarray(
        np.concatenate([Wt[:, :, :128], Wt[:, :, 128:]], axis=0).astype(npdt)
    )  # [128, 9, 128]; rows 0-63 ci for co-low, 64-127 ci for co-high
    return xpad, wsb


def _run(x, W, trace=False, **cfg):
    global _COMPILED, _CFG
    from concourse.bass_utils import run_bass_kernel_spmd

    if _COMPILED is None or cfg != _CFG:
        _COMPILED = _build(**cfg)
        _CFG = cfg
    nc = _COMPILED
    dt16 = cfg.get("dt16", True)
    grp = cfg.get("grp", 2)
    xs, wsb = _prep(x, W, dt16=dt16)
    in_maps = [
        {"x": np.ascontiguousarray(xs[c * BPC : (c + 1) * BPC]), "w": wsb}
        for c in range(NCORES)
    ]
    res = run_bass_kernel_spmd(
        nc, in_maps, core_ids=list(range(NCORES)), trace=trace
    )
    ys = np.concatenate(
        [np.asarray(res.results[c]["y"], dtype=np.float32) for c in range(NCORES)],
        axis=0,
    )  # [B, ngrp, 128, 2*grp, 2, WI]; tile t = 2*g + half
    ngrp = NHT // grp
    ys = ys.reshape(B, ngrp, 128, grp, 2, 2, WI)
    # -> [b, half, co128, hg, g, hh, wi]
    ys = np.ascontiguousarray(ys.transpose(0, 4, 2, 1, 3, 5, 6)).reshape(
        B, CO, H, WI
    )
    out = (
        ys.reshape(B, WC1, O, H, W_, IC0)
        .transpose(0, 2, 3, 4, 5, 1)
        .astype(np.float32)
    )
    return np.ascontiguousarray(out), res


def kernel(**inputs) -> np.ndarray:
    return _run(inputs["x"], inputs["W"])[0]
